# revision 1
# baseline (speedup 1.0000x reference)
"""Cross-attention 1d kernel for Trainium2 (Bass/Tile), SPMD over 8 NeuronCores.

Problem (hardcoded shapes): N=4, C=512, L=2048, H=8, D=64.
  out_a = out_a_w @ attn(a_norm -> b_norm) + out_a_b + a
  out_b = out_b_w @ attn(b_norm -> a_norm) + out_b_b + b

Sharding: 8 cores = 4 samples x 2 directions (a->b, b->a). Each core computes
one full [512, 2048] output tensor: GroupNorm(1) of both operands, its
direction's q projection + the other side's k/v projections, all 8 heads of
attention, and the output projection + residual. No cross-core communication;
host only slices/transposes weights and stacks the 8 results.

Per-core dataflow (all matmuls bf16 with fp32 PSUM accumulation):
  - GN stats: DVE free-axis reduce + ACT Square accum -> per-partition sums,
    then tiny ones-matmuls for the cross-partition reduce + broadcast.
  - q,k in [c, L] layout (c on partitions); v produced directly transposed
    [L, c] by swapping matmul operands (lhsT = yn tile, rhs = wv^T).
  - Attention per head-pair (heads 2p, 2p+1 live in partitions 0:64 / 64:128
    of channel-chunk p): per (k-tile, q-512-chunk) compute transposed scores
    for both heads into a double-buffered PSUM tile [128, 2heads, 512q]
    (row-tiled, concurrent on PE), exp in one ACT op (no max subtraction --
    scores are bounded ~|1|), then attn@v with v augmented by 64 replicated
    ones-columns so the softmax denominator lands broadcast across PSUM
    partitions 64:128 for free.
  - Normalize with reciprocal_approx_accurate + multiply while copying to the
    [c, L] attention-output buffer; out-projection + bias + residual fused.
"""

import sys

sys.path.insert(0, "/opt/trn_rl_repo")

import numpy as np
import ml_dtypes

import concourse.bass as bass
import concourse.tile as tile
from concourse import bacc, mybir
from concourse.bass import ts
from concourse.bass_utils import run_bass_kernel_spmd

F32 = mybir.dt.float32
BF16 = mybir.dt.bfloat16
AF = mybir.ActivationFunctionType
ALU = mybir.AluOpType

N, C, L, H = 4, 512, 2048, 8
D = C // H
EPS = 1e-5
SCALE = float(D) ** -0.5
P = 128
CO = C // P          # 4 channel chunks
LC = L // 512        # 4 column chunks of 512
LT = L // P          # 16 position tiles of 128
QH = 4               # q processed in quarters of 512 per head-pair sweep
QW = L // QH

BF16_NP = ml_dtypes.bfloat16


def _build_module():
    nc = bacc.Bacc("TRN2", target_bir_lowering=False, debug=False, num_devices=8)

    def din(name, shape, dt=F32):
        return nc.dram_tensor(name, list(shape), dt, kind="ExternalInput")

    x_d = din("x", (C, L))            # query-side input (residual side)
    y_d = din("y", (C, L))            # key/value-side input
    gnx_w = din("gnx_w", (C,))
    gnx_b = din("gnx_b", (C,))
    gny_w = din("gny_w", (C,))
    gny_b = din("gny_b", (C,))
    wqT_d = din("wqT", (C, C), BF16)  # wq.T  : [c_in, c_out]
    wkT_d = din("wkT", (C, C), BF16)
    wvT_d = din("wvT", (C, C), BF16)
    woT_d = din("woT", (C, C), BF16)
    bq_d = din("bq", (C,))
    bk_d = din("bk", (C,))
    bv_d = din("bv", (C,))
    bo_d = din("bo", (C,))
    out_d = nc.dram_tensor("out", [C, L], F32, kind="ExternalOutput")

    inv_cnt = 1.0 / float(C * L)

    with tile.TileContext(nc) as tc:
        with (
            tc.tile_pool(name="persist", bufs=1) as pp,
            tc.tile_pool(name="small", bufs=1) as sp,
        ):
            # ---- persistent tiles (~98 KB/partition) ----
            q_sb = pp.tile([P, CO, L], BF16)         # q * scale + bq      16K
            k_sb = pp.tile([P, CO, L], BF16)         # k + bk              16K
            vaug = pp.tile([P, LT, H, P], BF16)      # [l, lt, h, 64v|64one] 32K
            attn = pp.tile([P, CO, L], BF16)         # attention out [c,L] 16K
            wqT = pp.tile([P, CO, C], BF16)          # 4K each
            wkT = pp.tile([P, CO, C], BF16)
            wvT = pp.tile([P, CO, C], BF16)
            woT = pp.tile([P, CO, C], BF16)

            ones_col = sp.tile([P, 1], F32)
            ones_row = sp.tile([1, P], F32)
            nc.vector.memset(ones_col[:], 1.0)
            nc.vector.memset(ones_row[:], 1.0)
            bq_pc = sp.tile([P, CO], F32)
            bk_pc = sp.tile([P, CO], F32)
            bo_pc = sp.tile([P, CO], F32)
            bv_row = sp.tile([1, C], F32)
            bv_bc = sp.tile([P, C], F32)
            # gn affine vectors, preloaded as [P, CO]
            gnw_y_pc = sp.tile([P, CO], F32)
            gnb_y_pc = sp.tile([P, CO], F32)
            gnw_x_pc = sp.tile([P, CO], F32)
            gnb_x_pc = sp.tile([P, CO], F32)
            # ones half of v_aug, set once
            nc.gpsimd.memset(vaug[:, :, :, D:P], 1.0)

            with (
                tc.tile_pool(name="norm", bufs=1) as npool,
                tc.tile_pool(name="ps_qkv", bufs=2, space="PSUM") as psQ,
            ):
                yn = npool.tile([P, CO, L], BF16)
                xn = npool.tile([P, CO, L], BF16)

                with (
                    tc.tile_pool(name="gn_scr", bufs=2) as gsp,
                    tc.tile_pool(name="psA", bufs=2, space="PSUM") as psA,
                ):
                    def gn_scale_bias(src_sb, w_d, b_d, pref):
                        """[P,CO] scale/bias tiles: x_norm = x*scale + bias."""
                        st = sp.tile([P, 2], F32, tag=f"{pref}_st")
                        parts = gsp.tile([P, CO], F32, tag="gn_parts")
                        for co in range(CO):
                            nc.vector.tensor_reduce(parts[:, co:co + 1],
                                                    src_sb[:, co, :],
                                                    axis=mybir.AxisListType.X,
                                                    op=ALU.add)
                        nc.vector.tensor_reduce(st[:, 0:1], parts[:],
                                                axis=mybir.AxisListType.X,
                                                op=ALU.add)
                        sqp = gsp.tile([P, CO], F32, tag="gn_sqp")
                        for co in range(CO):
                            scr = gsp.tile([P, L], BF16, tag="gn_scr")
                            nc.scalar.activation(scr[:], src_sb[:, co, :],
                                                 AF.Square,
                                                 accum_out=sqp[:, co:co + 1])
                        nc.vector.tensor_reduce(st[:, 1:2], sqp[:],
                                                axis=mybir.AxisListType.X,
                                                op=ALU.add)
                        # cross-partition reduce then broadcast back, via PE
                        tot_p = psA.tile([1, 2], F32, tag="gn_totp")
                        nc.tensor.matmul(tot_p[:], ones_col[:], st[:],
                                         start=True, stop=True)
                        t12 = sp.tile([1, 2], F32, tag=f"{pref}_t12")
                        nc.scalar.copy(t12[:], tot_p[:])
                        bc_p = psA.tile([P, 2], F32, tag="gn_bcp")
                        nc.tensor.matmul(bc_p[:], ones_row[:], t12[:],
                                         start=True, stop=True)
                        tot = sp.tile([P, 2], F32, tag=f"{pref}_tot")
                        nc.vector.tensor_copy(tot[:], bc_p[:])

                        mu = sp.tile([P, 1], F32, tag=f"{pref}_mu")
                        nc.vector.tensor_scalar(mu[:], tot[:, 0:1], inv_cnt, 0.0,
                                                op0=ALU.mult, op1=ALU.add)
                        var = sp.tile([P, 1], F32, tag=f"{pref}_var")
                        # var + eps = (E[x^2] + eps) - mu^2
                        nc.vector.tensor_scalar(var[:], tot[:, 1:2], inv_cnt, EPS,
                                                op0=ALU.mult, op1=ALU.add)
                        musq = sp.tile([P, 1], F32, tag=f"{pref}_musq")
                        nc.vector.tensor_scalar(musq[:], mu[:], mu[:], 0.0,
                                                op0=ALU.mult, op1=ALU.add)
                        nc.vector.tensor_tensor(var[:], var[:], musq[:],
                                                ALU.subtract)
                        std = sp.tile([P, 1], F32, tag=f"{pref}_std")
                        nc.scalar.activation(std[:], var[:], AF.Sqrt)
                        rstd = sp.tile([P, 1], F32, tag=f"{pref}_rstd")
                        nc.vector.reciprocal(rstd[:], std[:])
                        nmu = sp.tile([P, 1], F32, tag=f"{pref}_nmu")
                        nc.vector.tensor_scalar(nmu[:], mu[:], -1.0, 0.0,
                                                op0=ALU.mult, op1=ALU.add)

                        w_pc, b_pc = w_d, b_d
                        scale = sp.tile([P, CO], F32, tag=f"{pref}_scale")
                        bias = sp.tile([P, CO], F32, tag=f"{pref}_bias")
                        nc.vector.tensor_scalar(scale[:], w_pc[:], rstd[:], 0.0,
                                                op0=ALU.mult, op1=ALU.add)
                        nc.vector.scalar_tensor_tensor(bias[:], scale[:], nmu[:],
                                                       b_pc[:],
                                                       op0=ALU.mult, op1=ALU.add)
                        return scale, bias

                    with tc.tile_pool(name="ph_y", bufs=1) as yp:
                        y_sb = yp.tile([P, CO, L], F32)
                        for co in range(CO):
                            nc.sync.dma_start(
                                y_sb[:, co, :],
                                y_d[:].rearrange("(co p) l -> p co l", p=P)[:, co, :])
                        # everything else queues on sync after the y chunks
                        for dr, t in ((gny_w, gnw_y_pc), (gny_b, gnb_y_pc),
                                      (gnx_w, gnw_x_pc), (gnx_b, gnb_x_pc),
                                      (bq_d, bq_pc), (bk_d, bk_pc), (bo_d, bo_pc)):
                            nc.sync.dma_start(
                                t[:], dr[:].rearrange("(co p) -> p co", p=P))
                        nc.sync.dma_start(
                            bv_row[:], bv_d[:].rearrange("(a c) -> a c", a=1))
                        nc.gpsimd.partition_broadcast(bv_bc[:], bv_row[:])
                        for dr, t in ((wvT_d, wvT), (wkT_d, wkT),
                                      (wqT_d, wqT), (woT_d, woT)):
                            nc.sync.dma_start(
                                t[:], dr[:].rearrange("(ko p) o -> p ko o", p=P))
                        s_y, b_y = gn_scale_bias(y_sb, gnw_y_pc, gnb_y_pc, "y")
                        for co in range(CO):
                            nc.vector.tensor_scalar(yn[:, co, :], y_sb[:, co, :],
                                                    s_y[:, co:co + 1],
                                                    b_y[:, co:co + 1],
                                                    op0=ALU.mult, op1=ALU.add)

                    with tc.tile_pool(name="ph_x", bufs=1) as xp:
                        x_sb = xp.tile([P, CO, L], F32)
                        for co in range(CO):
                            nc.sync.dma_start(
                                x_sb[:, co, :],
                                x_d[:].rearrange("(co p) l -> p co l", p=P)[:, co, :])
                        s_x, b_x = gn_scale_bias(x_sb, gnw_x_pc, gnb_x_pc, "x")
                        for co in range(CO):
                            nc.vector.tensor_scalar(xn[:, co, :], x_sb[:, co, :],
                                                    s_x[:, co:co + 1],
                                                    b_x[:, co:co + 1],
                                                    op0=ALU.mult, op1=ALU.add)

                    # vT = (wv @ yn)^T + bv -> vaug[:, lt, h, 0:64]
                    for lt in range(LT):
                        vp = psQ.tile([P, C], F32, tag="mm")
                        for ko in range(CO):
                            nc.tensor.matmul(vp[:], yn[:, ko, ts(lt, P)],
                                             wvT[:, ko, :],
                                             start=(ko == 0), stop=(ko == CO - 1))
                        nc.vector.tensor_tensor(
                            vaug[:, lt, :, 0:D],
                            vp[:].rearrange("p (h d) -> p h d", d=D),
                            bv_bc[:].rearrange("p (h d) -> p h d", d=D),
                            ALU.add)


                # ======== attention, with per-pair k/q projections ========
                with (
                    tc.tile_pool(name="ps_sc", bufs=2, space="PSUM") as ps_sc,
                    tc.tile_pool(name="ps_out", bufs=1, space="PSUM") as ps_out,
                    tc.tile_pool(name="pt_pool", bufs=6) as ptp,
                    tc.tile_pool(name="tail", bufs=2) as tlp,
                ):
                    def qkv_mm(dst_sb, wT, src_sb, mo, bias_pc, scale2):
                        """dst[:, mo, :] = (wT.T @ src + bias) * scale2, by lc."""
                        for lc in range(LC):
                            mmp = psQ.tile([P, 512], F32, tag="mm")
                            for ko in range(CO):
                                nc.tensor.matmul(mmp, wT[:, ko, ts(mo, P)],
                                                 src_sb[:, ko, ts(lc, 512)],
                                                 start=(ko == 0),
                                                 stop=(ko == CO - 1))
                            nc.vector.tensor_scalar(dst_sb[:, mo, ts(lc, 512)],
                                                    mmp, bias_pc[:, mo:mo + 1],
                                                    scale2,
                                                    op0=ALU.add, op1=ALU.mult)

                    for p in range(CO):      # head pair p -> heads 2p, 2p+1
                        qkv_mm(k_sb, wkT, yn, p, bk_pc, 1.0)
                        qkv_mm(q_sb, wqT, xn, p, bq_pc, SCALE)
                        for qq in range(QH):
                            qs = qq * QW
                            oA = ps_out.tile([P, QW], F32, tag="oA")
                            oB = ps_out.tile([P, QW], F32, tag="oB")
                            for kt in range(LT):
                                scp = ps_sc.tile([P, 2, QW], F32, tag="sc")
                                nc.tensor.matmul(scp[:, 0, :],
                                                 k_sb[0:D, p, ts(kt, P)],
                                                 q_sb[0:D, p, qs:qs + QW],
                                                 start=True, stop=True)
                                nc.tensor.matmul(scp[:, 1, :],
                                                 k_sb[D:P, p, ts(kt, P)],
                                                 q_sb[D:P, p, qs:qs + QW],
                                                 start=True, stop=True)
                                pt = ptp.tile([P, 2, QW], BF16, tag="pt")
                                nc.scalar.activation(pt[:], scp[:], AF.Exp)
                                nc.tensor.matmul(
                                    oA[:], vaug[:, kt, 2 * p, :], pt[:, 0, :],
                                    start=(kt == 0), stop=(kt == LT - 1))
                                nc.tensor.matmul(
                                    oB[:], vaug[:, kt, 2 * p + 1, :], pt[:, 1, :],
                                    start=(kt == 0), stop=(kt == LT - 1))
                            for hx, ops in ((0, oA), (1, oB)):
                                h = 2 * p + hx
                                # single copy releases the PSUM accumulator
                                # ASAP; the rest of the tail runs off SBUF
                                # and overlaps the next sweep.
                                t_all = tlp.tile([P, QW], F32, tag="t")
                                nc.vector.tensor_copy(t_all[:], ops[:])
                                # reciprocal_approx (custom DVE op) needs a
                                # base-partition-0 input; shift S down first.
                                s_sb = tlp.tile([D, QW], F32, tag="s")
                                nc.vector.tensor_copy(s_sb[:], t_all[D:P, :])
                                r_sb = tlp.tile([D, QW], F32, tag="r")
                                scr = tlp.tile([D, QW], F32, tag="rs")
                                nc.vector.reciprocal_approx_accurate(
                                    r_sb[:], s_sb[:], scr[:])
                                lo = D * (h % 2)
                                nc.vector.tensor_tensor(
                                    attn[lo:lo + D, h // 2, qs:qs + QW],
                                    t_all[0:D, :], r_sb[:], ALU.mult)

                    # ====== out projection + residual (psQ slots; overlaps
                    # the tail of the attention pair loop) ======
                    with (
                        tc.tile_pool(name="outsb", bufs=3) as osp,
                        tc.tile_pool(name="xre", bufs=3) as xrp,
                    ):
                        for lc in range(LC):
                            for mo in range(CO):
                                op = psQ.tile([P, 512], F32, tag="mm")
                                for ko in range(CO):
                                    nc.tensor.matmul(op[:], woT[:, ko, ts(mo, P)],
                                                     attn[:, ko, ts(lc, 512)],
                                                     start=(ko == 0),
                                                     stop=(ko == CO - 1))
                                xr = xrp.tile([P, 512], F32, tag="xr")
                                nc.sync.dma_start(
                                    xr[:],
                                    x_d[:].rearrange("(mo p) l -> p mo l", p=P)[:, mo, ts(lc, 512)])
                                o_sb = osp.tile([P, 512], F32, tag="osb")
                                nc.vector.scalar_tensor_tensor(
                                    o_sb[:], op[:], bo_pc[:, mo:mo + 1], xr[:],
                                    op0=ALU.add, op1=ALU.add)
                                nc.sync.dma_start(
                                    out_d[:].rearrange("(mo p) l -> p mo l", p=P)[:, mo, ts(lc, 512)],
                                    o_sb[:])

    nc.compile()
    return nc


_NC_CACHE = None


def _get_module():
    global _NC_CACHE
    if _NC_CACHE is None:
        _NC_CACHE = _build_module()
    return _NC_CACHE


def _core_inputs(x, y, gnx_w, gnx_b, gny_w, gny_b, qw_q, qb_q, qw_kv, qb_kv, ow, ob):
    bf = lambda a: np.ascontiguousarray(np.asarray(a).T).astype(BF16_NP)
    return {
        "x": np.ascontiguousarray(x, dtype=np.float32),
        "y": np.ascontiguousarray(y, dtype=np.float32),
        "gnx_w": np.asarray(gnx_w, np.float32), "gnx_b": np.asarray(gnx_b, np.float32),
        "gny_w": np.asarray(gny_w, np.float32), "gny_b": np.asarray(gny_b, np.float32),
        "wqT": bf(qw_q[0:C]), "bq": np.asarray(qb_q[0:C], np.float32),
        "wkT": bf(qw_kv[C:2 * C]), "bk": np.asarray(qb_kv[C:2 * C], np.float32),
        "wvT": bf(qw_kv[2 * C:3 * C]), "bv": np.asarray(qb_kv[2 * C:3 * C], np.float32),
        "woT": bf(ow), "bo": np.asarray(ob, np.float32),
    }


def kernel(a, b, gn_a_w, gn_a_b, gn_b_w, gn_b_b,
           qkv_a_w, qkv_a_b, qkv_b_w, qkv_b_b,
           out_a_w, out_a_b, out_b_w, out_b_b):
    a = np.asarray(a); b = np.asarray(b)
    nc = _get_module()
    in_maps = []
    for s in range(N):
        # direction a->b : q from a, k/v from b, output -> out_a[s]
        in_maps.append(_core_inputs(a[s], b[s], gn_a_w, gn_a_b, gn_b_w, gn_b_b,
                                    qkv_a_w, qkv_a_b, qkv_b_w, qkv_b_b,
                                    out_a_w, out_a_b))
        # direction b->a : q from b, k/v from a, output -> out_b[s]
        in_maps.append(_core_inputs(b[s], a[s], gn_b_w, gn_b_b, gn_a_w, gn_a_b,
                                    qkv_b_w, qkv_b_b, qkv_a_w, qkv_a_b,
                                    out_b_w, out_b_b))
    res = run_bass_kernel_spmd(nc, in_maps, core_ids=list(range(2 * N)))
    out_a = np.stack([res.results[2 * s]["out"] for s in range(N)])
    out_b = np.stack([res.results[2 * s + 1]["out"] for s in range(N)])
    return out_a.astype(np.float32), out_b.astype(np.float32)



# revision 2
# speedup vs baseline: 1.4173x; 1.4173x over previous
"""Cross-attention 1d kernel for Trainium2 (Bass/Tile), SPMD over 8 NeuronCores.

Problem (hardcoded shapes): N=4, C=512, L=2048, H=8, D=64.
  out_x = out_w @ attn(x_norm -> y_norm) + out_b + x   (per core: one (sample,
  direction) pair; 8 cores = 4 samples x 2 directions; no cross-core comms.)

v2 design (vs v1 365us):
  - All projection + attention matmuls in fp8e4m3 with perf_mode=DoubleRow:
    contraction pairs ride the free dims ([K,2,M] lhsT / [K,2,N] rhs), halving
    PE streaming cost twice over (0.5 cyc/row fp8-DR in the cost model).
    Weights are pre-scaled x32 host-side to dodge fp8 subnormals; the 1/32
    rides the (required anyway) PSUM-evacuation ops.
  - Softmax exp is the true bottleneck (H*L*L/128 = 262k engine-columns).
    Split across BOTH pointwise engines: ACT runs real Exp straight to fp8;
    DVE runs Schraudolph's bit-trick exp (score*8*log2e + 56 - 0.35 written
    as uint8 == the fp8e4m3 bit pattern of e^score, max ~9% per-element err,
    irrelevant after averaging over 2048 softmax terms).
  - GroupNorm stats via one-pass bn_stats/bn_aggr (DVE), normalize+fp8-quant
    on GpSimd (frees DVE), cross-partition reduce via tiny ones-matmuls.
  - Softmax denominator via ones-augmented v columns (v|ones swapped per head
    parity so numerators land on the partition half the [c,L] attention
    layout needs); normalize = reciprocal + one multiply per (head, q-chunk).
  - Out-projection stays bf16 (exact weights, 3-term fused evac w/ residual).
"""

import sys

sys.path.insert(0, "/opt/trn_rl_repo")

import numpy as np
import ml_dtypes

import concourse.bass as bass
import concourse.tile as tile
from concourse import bacc, mybir
from concourse.bass import ts
from concourse.bass_utils import run_bass_kernel_spmd

F32 = mybir.dt.float32
BF16 = mybir.dt.bfloat16
FP8 = mybir.dt.float8e4
U8 = mybir.dt.uint8
AF = mybir.ActivationFunctionType
ALU = mybir.AluOpType
DR = mybir.MatmulPerfMode.DoubleRow

N, C, L, H = 4, 512, 2048, 8
D = C // H
EPS = 1e-5
SCALE = float(D) ** -0.5
P = 128
CO = C // P          # 4 channel chunks
LT = L // P          # 16 position tiles of 128
TP = LT // 2         # 8 kt pairs
QC = L // 512        # 4 q chunks of 512
WS = 32.0            # host-side weight pre-scale (fp8 subnormal dodge)

# Schraudolph exp in fp8e4m3 bit space: byte = 8*(x*log2e + 7) - 0.35
LOG2E = 1.4426950408889634
A8 = 8.0 * LOG2E
B8 = 56.0 - 0.35

# exp engine split: ACT true Exp vs DVE Schraudolph, ~59% on ACT
ACT_NUM, ACT_DEN = 38, 64
LEAD = 2             # attn@v trails scores by this many tp slots (global)

BF16_NP = ml_dtypes.bfloat16
FP8_NP = ml_dtypes.float8_e4m3


def _build_module():
    nc = bacc.Bacc("TRN2", target_bir_lowering=False, debug=False, num_devices=8)

    def din(name, shape, dt=F32):
        return nc.dram_tensor(name, list(shape), dt, kind="ExternalInput")

    x_d = din("x", (C, L))            # query-side input (residual, f32)
    xh_d = din("xh", (C, L), BF16)    # bf16 copies: stats/normalize inputs
    yh_d = din("yh", (C, L), BF16)
    gnx_w = din("gnx_w", (C,))
    gnx_b = din("gnx_b", (C,))
    gny_w = din("gny_w", (C,))
    gny_b = din("gny_b", (C,))
    wqT_d = din("wqT", (P, 2 * 2 * C), FP8)        # [p,a,j,c_out] pair-split wT, x32
    wkT_d = din("wkT", (P, 2 * 2 * C), FP8)
    wvT_d = din("wvT", (P, 2 * 2 * C), FP8)        # [p,a,j,c_out], x32
    woT_d = din("woT", (P, CO * C), BF16)          # [p,ko,o] plain transpose
    bq_d = din("bq", (P, CO))         # natural (co p) rows, pre-scaled by SCALE
    bk_d = din("bk", (P, CO))         # natural (co p) rows
    bv_d = din("bv", (C,))
    bo_d = din("bo", (P, CO))         # natural (mo p) rows
    out_d = nc.dram_tensor("out", [C, L], F32, kind="ExternalOutput")

    with tile.TileContext(nc) as tc:
        with (
            tc.tile_pool(name="persist", bufs=1) as pp,
            tc.tile_pool(name="small", bufs=1) as sp,
        ):
            # ---- persistent tiles ----
            q_sb = pp.tile([P, CO, L], FP8)          # [p, co, pos] 8K
            k_sb = pp.tile([P, CO, L], FP8)          # 8K
            vaug = pp.tile([P, LT, H, P], FP8)       # [k, lt, h, v|ones] 16K
            attn = pp.tile([P, CO, L], BF16)         # attention out [c,L] 16K
            wqT = pp.tile([P, 2, 2, CO, P], FP8)     # [p,a,j,mo,o] 2K
            wkT = pp.tile([P, 2, 2, CO, P], FP8)
            wvT = pp.tile([P, 2, 2, C], FP8)
            woT = pp.tile([P, CO, C], BF16)          # 4K

            ones_col = sp.tile([P, 1], F32)
            ones_row = sp.tile([1, P], F32)
            nc.vector.memset(ones_col[:], 1.0)
            nc.vector.memset(ones_row[:], 1.0)
            bq_pc = sp.tile([P, CO], F32)
            bk_pc = sp.tile([P, CO], F32)
            bo_pc = sp.tile([P, CO], F32)
            bv_row = sp.tile([1, C], F32)
            bv_bc = sp.tile([P, C], F32)
            gnw_y_pc = sp.tile([P, CO], F32)
            gnb_y_pc = sp.tile([P, CO], F32)
            gnw_x_pc = sp.tile([P, CO], F32)
            gnb_x_pc = sp.tile([P, CO], F32)

            with tc.tile_pool(name="norm", bufs=1) as npool:
                yn = npool.tile([P, 2, 2, L], FP8)   # [p, a, j, pos] 8K
                xn = npool.tile([P, 2, 2, L], FP8)
                x_sb = npool.tile([P, CO, L], BF16)

                with (
                    tc.tile_pool(name="ps_qkv", bufs=3, space="PSUM") as psQ,
                    tc.tile_pool(name="gn_scr", bufs=2) as gsp,
                    tc.tile_pool(name="psA", bufs=2, space="PSUM") as psA,
                ):
                    def gn_tail(st, w_pc, b_pc, pref, inv):
                        """st [P,2] per-partition (1st, 2nd) moments -> scale/bias;
                        inv converts cross-partition totals to mu / E[x^2]."""
                        tot_p = psA.tile([1, 2], F32, tag="gn_totp")
                        nc.tensor.matmul(tot_p[:], ones_col[:], st[:],
                                         start=True, stop=True)
                        t12 = sp.tile([1, 2], F32, tag=f"{pref}_t12")
                        nc.scalar.copy(t12[:], tot_p[:])
                        bc_p = psA.tile([P, 2], F32, tag="gn_bcp")
                        nc.tensor.matmul(bc_p[:], ones_row[:], t12[:],
                                         start=True, stop=True)
                        mu = sp.tile([P, 1], F32, tag=f"{pref}_mu")
                        nc.vector.tensor_scalar(mu[:], bc_p[:, 0:1], inv, 0.0,
                                                op0=ALU.mult, op1=ALU.add)
                        var = sp.tile([P, 1], F32, tag=f"{pref}_var")
                        nc.vector.tensor_scalar(var[:], bc_p[:, 1:2], inv, EPS,
                                                op0=ALU.mult, op1=ALU.add)
                        musq2 = sp.tile([P, 1], F32, tag=f"{pref}_musq2")
                        nc.vector.tensor_tensor(musq2[:], mu[:], mu[:], ALU.mult)
                        nc.vector.tensor_tensor(var[:], var[:], musq2[:],
                                                ALU.subtract)
                        std = sp.tile([P, 1], F32, tag=f"{pref}_std")
                        nc.scalar.activation(std[:], var[:], AF.Sqrt)
                        rstd = sp.tile([P, 1], F32, tag=f"{pref}_rstd")
                        nc.vector.reciprocal(rstd[:], std[:])
                        nmu = sp.tile([P, 1], F32, tag=f"{pref}_nmu")
                        nc.vector.tensor_scalar(nmu[:], mu[:], -1.0, 0.0,
                                                op0=ALU.mult, op1=ALU.add)
                        scale = sp.tile([P, CO], F32, tag=f"{pref}_scale")
                        bias = sp.tile([P, CO], F32, tag=f"{pref}_bias")
                        nc.vector.tensor_scalar(scale[:], w_pc[:], rstd[:], 0.0,
                                                op0=ALU.mult, op1=ALU.add)
                        nc.vector.scalar_tensor_tensor(bias[:], scale[:], nmu[:],
                                                       b_pc[:],
                                                       op0=ALU.mult, op1=ALU.add)
                        return scale, bias

                    def gn_scale_bias(src_sb, w_pc, b_pc, pref):
                        """y path: one-pass bn_stats (DVE) -> st -> gn_tail."""
                        bns = gsp.tile([P, CO, 4, 6], F32, tag="gn_bns")
                        for co in range(CO):
                            for s4 in range(4):
                                nc.vector.bn_stats(
                                    bns[:, co, s4, :],
                                    src_sb[:, co, ts(s4, 512)])
                        bna = gsp.tile([P, 2], F32, tag="gn_bna")
                        nc.vector.bn_aggr(bna[:], bns[:].rearrange("p a b c -> p (a b c)"))
                        st = sp.tile([P, 2], F32, tag=f"{pref}_st")
                        musq = gsp.tile([P, 1], F32, tag="gn_musq")
                        nc.vector.tensor_tensor(musq[:], bna[:, 0:1], bna[:, 0:1],
                                                ALU.mult)
                        nc.vector.tensor_copy(st[:, 0:1], bna[:, 0:1])
                        nc.vector.tensor_tensor(st[:, 1:2], bna[:, 1:2], musq[:],
                                                ALU.add)
                        return gn_tail(st, w_pc, b_pc, pref, 1.0 / P)

                    def gn_from_sums(sump, sqp, w_pc, b_pc, pref):
                        """x path: per-partition sums + squared-sums -> gn_tail."""
                        st = sp.tile([P, 2], F32, tag=f"{pref}_st")
                        nc.vector.tensor_reduce(st[:, 0:1], sump[:],
                                                axis=mybir.AxisListType.X,
                                                op=ALU.add)
                        nc.vector.tensor_reduce(st[:, 1:2], sqp[:],
                                                axis=mybir.AxisListType.X,
                                                op=ALU.add)
                        return gn_tail(st, w_pc, b_pc, pref, 1.0 / float(C * L))

                    with tc.tile_pool(name="io", bufs=1) as iop:
                        y_sb = iop.tile([P, CO, L], BF16)
                        for co in range(CO):
                            nc.sync.dma_start(
                                y_sb[:, co, :],
                                yh_d[:].rearrange("(co p) l -> p co l", p=P)[:, co, :])
                        for co in range(CO):
                            # second hwdge queue so x streams in parallel with y
                            nc.gpsimd.dma_start(
                                x_sb[:, co, :],
                                xh_d[:].rearrange("(co p) l -> p co l", p=P)[:, co, :])
                        for dr, t in ((gny_w, gnw_y_pc), (gny_b, gnb_y_pc),
                                      (gnx_w, gnw_x_pc), (gnx_b, gnb_x_pc)):
                            nc.sync.dma_start(
                                t[:], dr[:].rearrange("(co p) -> p co", p=P))
                        for dr, t in ((bq_d, bq_pc), (bk_d, bk_pc), (bo_d, bo_pc)):
                            nc.sync.dma_start(t[:], dr[:])
                        nc.sync.dma_start(
                            bv_row[:], bv_d[:].rearrange("(a c) -> a c", a=1))
                        nc.gpsimd.partition_broadcast(bv_bc[:], bv_row[:])
                        # ones half of v_aug: even heads cols D:P, odd 0:D
                        nc.gpsimd.memset(vaug[:, :, 0:H:2, D:P], 1.0)
                        nc.gpsimd.memset(vaug[:, :, 1:H:2, 0:D], 1.0)
                        # bv (x32, bf16) rides the v matmul as a rank-1 update
                        bv32_row = sp.tile([1, C], BF16)
                        nc.gpsimd.tensor_scalar(bv32_row[:], bv_row[:], WS, 0.0,
                                                op0=ALU.mult, op1=ALU.add)
                        ones_1 = sp.tile([1, P], BF16)
                        nc.vector.memset(ones_1[:], 1.0)
                        nc.sync.dma_start(
                            wvT[:], wvT_d[:].rearrange("p (a j c) -> p a j c", a=2, j=2))
                        nc.sync.dma_start(
                            wkT[:], wkT_d[:].rearrange("p (a j t o) -> p a j t o",
                                                       a=2, j=2, t=CO))
                        nc.sync.dma_start(
                            wqT[:], wqT_d[:].rearrange("p (a j t o) -> p a j t o",
                                                       a=2, j=2, t=CO))
                        nc.sync.dma_start(
                            woT[:], woT_d[:].rearrange("p (ko o) -> p ko o", ko=CO))

                        # y stats first (bn_stats on DVE; tiny ACT tail)
                        s_y, b_y = gn_scale_bias(y_sb, gnw_y_pc, gnb_y_pc, "y")
                        for a in range(2):
                            for j in range(2):
                                co = 2 * a + j
                                nc.vector.tensor_scalar(
                                    yn[:, a, j, :], y_sb[:, co, :],
                                    s_y[:, co:co + 1], b_y[:, co:co + 1],
                                    op0=ALU.mult, op1=ALU.add)
                        # x stats: ACT squares || DVE sums (behind yn norms)
                        sqp = gsp.tile([P, CO], F32, tag="x_sqp")
                        sump = gsp.tile([P, CO], F32, tag="x_sump")
                        for co in range(CO):
                            sq_scr = gsp.tile([P, L], BF16, tag="x_sqscr")
                            nc.scalar.activation(sq_scr[:], x_sb[:, co, :],
                                                 AF.Square,
                                                 accum_out=sqp[:, co:co + 1])
                        for co in range(CO):
                            nc.vector.tensor_reduce(sump[:, co:co + 1],
                                                    x_sb[:, co, :],
                                                    axis=mybir.AxisListType.X,
                                                    op=ALU.add)
                        s_x, b_x = gn_from_sums(sump, sqp, gnw_x_pc, gnb_x_pc, "x")
                        for a in range(2):
                            for j in range(2):
                                co = 2 * a + j
                                nc.vector.tensor_scalar(
                                    xn[:, a, j, :], x_sb[:, co, :],
                                    s_x[:, co:co + 1], b_x[:, co:co + 1],
                                    op0=ALU.mult, op1=ALU.add)

                        # vT = (wv32 @ yn + 32 bv)^T / 32 -> vaug v-halves (ACT)
                        for lt in range(LT):
                            vp = psQ.tile([P, C], F32, tag="mm")
                            for a in range(2):
                                nc.tensor.matmul(vp[:], yn[:, a, :, ts(lt, P)],
                                                 wvT[:, a, :, :],
                                                 start=(a == 0), stop=False,
                                                 perf_mode=DR)
                            nc.tensor.matmul(vp[:], ones_1[:], bv32_row[:],
                                             start=False, stop=True)
                            vp_h = vp[:].rearrange("p (h d) -> p h d", d=D)
                            nc.scalar.activation(vaug[:, lt, 0:H:2, 0:D],
                                                 vp_h[:, 0:H:2, :], AF.Copy,
                                                 scale=1.0 / WS)
                            nc.scalar.activation(vaug[:, lt, 1:H:2, D:P],
                                                 vp_h[:, 1:H:2, :], AF.Copy,
                                                 scale=1.0 / WS)




                # ======== attention ========
                with (
                    tc.tile_pool(name="ps_sc", bufs=3, space="PSUM") as ps_sc,
                    tc.tile_pool(name="ps_out", bufs=2, space="PSUM") as ps_out,
                    tc.tile_pool(name="pt_pool", bufs=4) as ptp,
                    tc.tile_pool(name="tail", bufs=2) as tlp,
                ):
                    with (
                        tc.tile_pool(name="outsb", bufs=3) as osp,
                        tc.tile_pool(name="xre", bufs=3) as xrp,
                    ):
                        def outproj_chunk(lc, mo):
                            # out[:, mo, lc] = woT.T @ attn[:, :, lc] + bo + x
                            # (bf16 weights; evac ACT, residual add on Pool)
                            op = ps_out.tile([P, 512], F32, tag="oA",
                                             name=f"op_{lc}_{mo}")
                            for ko in range(CO):
                                nc.tensor.matmul(op[:], woT[:, ko, ts(mo, P)],
                                                 attn[:, ko, ts(lc, 512)],
                                                 start=(ko == 0),
                                                 stop=(ko == CO - 1))
                            o1 = osp.tile([P, 512], F32, tag="o1")
                            nc.scalar.activation(o1[:], op[:], AF.Identity,
                                                 bias=bo_pc[:, mo:mo + 1])
                            xr = xrp.tile([P, 512], F32, tag="xr")
                            nc.sync.dma_start(
                                xr[:],
                                x_d[:].rearrange("(mo p) l -> p mo l", p=P)[:, mo, ts(lc, 512)])
                            o_sb = osp.tile([P, 512], F32, tag="osb")
                            nc.gpsimd.tensor_tensor(o_sb[:], o1[:], xr[:], ALU.add)
                            nc.sync.dma_start(
                                out_d[:].rearrange("(mo p) l -> p mo l", p=P)[:, mo, ts(lc, 512)],
                                o_sb[:])

                        def emit_kproj(mo):
                            # k[:, mo, :] projected just before heads 2mo/2mo+1
                            for lc in range(QC):
                                mmt = ps_sc.tile([P, 2, 512], F32, tag="sc",
                                                 name=f"kmm_{mo}_{lc}")
                                for a in range(2):
                                    nc.tensor.matmul(
                                        mmt[:, 0, :], wkT[:, a, :, mo, :],
                                        yn[:, a, :, ts(lc, 512)],
                                        start=(a == 0), stop=(a == 1),
                                        perf_mode=DR)
                                nc.vector.tensor_scalar(
                                    k_sb[:, mo, ts(lc, 512)], mmt[:, 0, :],
                                    1.0 / WS, bk_pc[:, mo:mo + 1],
                                    op0=ALU.mult, op1=ALU.add)

                        def emit_qproj(lc):
                            # q[:, :, lc] projected just-in-time for its qc
                            for mo in range(CO):
                                mmt = ps_sc.tile([P, 2, 512], F32, tag="sc",
                                                 name=f"qmm_{lc}_{mo}")
                                for a in range(2):
                                    nc.tensor.matmul(
                                        mmt[:, 0, :], wqT[:, a, :, mo, :],
                                        xn[:, a, :, ts(lc, 512)],
                                        start=(a == 0), stop=(a == 1),
                                        perf_mode=DR)
                                nc.scalar.activation(
                                    q_sb[:, mo, ts(lc, 512)], mmt[:, 0, :],
                                    AF.Identity, bias=bq_pc[:, mo:mo + 1],
                                    scale=0.5 * SCALE / WS)

                        # Flat software pipeline over all (sweep, tp) slots:
                        # attn@v trails scores/exp by LEAD slots globally, so
                        # sweep boundaries don't bubble the exp engines.
                        sweeps = [(qc, h) for qc in range(QC) for h in range(H)]
                        nslots = len(sweeps) * TP
                        oAs = {}
                        out_todo = []

                        def emit_slot(i):
                            sw, tp = divmod(i, TP)
                            qc, h = sweeps[sw]
                            co_h = h // 2
                            lo = D * (h % 2)
                            scp = ps_sc.tile([P, 2, 512], F32, tag="sc")
                            qv = (q_sb[lo:lo + D, co_h, ts(qc, 512)]
                                  .rearrange("p (a k) -> p a k", a=1)
                                  .broadcast_to((D, 2, 512)))
                            for e in range(2):
                                kt = 2 * tp + e
                                kv = (k_sb[lo:lo + D, co_h, ts(kt, P)]
                                      .rearrange("p (a k) -> p a k", a=1)
                                      .broadcast_to((D, 2, P)))
                                nc.tensor.matmul(
                                    scp[:, e, :], kv, qv,
                                    start=True, stop=True, perf_mode=DR)
                            pt = ptp.tile([P, 2, 512], U8, tag="pt")
                            if (i * ACT_NUM) % ACT_DEN < ACT_NUM:
                                nc.scalar.activation(pt[:].bitcast(FP8),
                                                     scp[:], AF.Exp)
                            else:
                                nc.vector.tensor_scalar(pt[:], scp[:], A8, B8,
                                                        op0=ALU.mult, op1=ALU.add)
                            return pt

                        def emit_av(i, pt):
                            sw, tp = divmod(i, TP)
                            qc, h = sweeps[sw]
                            if tp == 0:
                                oAs[sw] = ps_out.tile([P, 512], F32, tag="oA", name=f"oA_{sw}")
                            nc.tensor.matmul(
                                oAs[sw][:], vaug[:, 2 * tp:2 * tp + 2, h, :],
                                pt[:].bitcast(FP8),
                                start=(tp == 0), stop=(tp == TP - 1),
                                perf_mode=DR)
                            if tp == TP - 1:
                                oA = oAs.pop(sw)
                                lo = D * (h % 2)
                                r = tlp.tile([D, 512], F32, tag="r")
                                nc.vector.reciprocal(r[:],
                                                     oA[P - D - lo:P - lo, :])
                                nc.vector.tensor_tensor(
                                    attn[lo:lo + D, h // 2, ts(qc, 512)],
                                    oA[lo:lo + D, :], r[:], ALU.mult)
                                if h == H - 1:
                                    out_todo.extend(
                                        (lambda lc=qc, mo=mo: outproj_chunk(lc, mo))
                                        for mo in range(CO))

                        pts = {}
                        for i in range(nslots):
                            sw, tp = divmod(i, TP)
                            if tp == 0 and sweeps[sw][0] == 0 \
                                    and sweeps[sw][1] % 2 == 0:
                                emit_kproj(sweeps[sw][1] // 2)
                            if tp == 0 and sweeps[sw][1] == 0:
                                emit_qproj(sweeps[sw][0])
                            pts[i] = emit_slot(i)
                            if i >= LEAD:
                                emit_av(i - LEAD, pts.pop(i - LEAD))
                            if out_todo and (i % TP) == 3:
                                out_todo.pop(0)()
                        for i in range(nslots - LEAD, nslots):
                            emit_av(i, pts.pop(i))
                        for fn in out_todo:
                            fn()

    nc.compile()
    return nc


_NC_CACHE = None


def _get_module():
    global _NC_CACHE
    if _NC_CACHE is None:
        _NC_CACHE = _build_module()
    return _NC_CACHE


def _pack_w8(w):
    """w [C_out, C_in] -> wT pair-split [p, a, j, c_out] fp8, x32."""
    a = (np.asarray(w, np.float64) * WS).T                 # [c_in, c_out]
    a = a.reshape(2, 2, P, C).transpose(2, 0, 1, 3)        # [p, a, j, c_out]
    return np.ascontiguousarray(a.reshape(P, -1)).astype(FP8_NP)


def _core_inputs(x, y, gnx_w, gnx_b, gny_w, gny_b, qw_q, qb_q, qw_kv, qb_kv, ow, ob):
    wq = qw_q[0:C]
    wk = qw_kv[C:2 * C]
    wv = qw_kv[2 * C:3 * C]
    bq = np.asarray(qb_q[0:C], np.float32)
    bk = np.asarray(qb_kv[C:2 * C], np.float32)
    bv = np.asarray(qb_kv[2 * C:3 * C], np.float32)
    return {
        "x": np.ascontiguousarray(x, dtype=np.float32),
        "xh": np.ascontiguousarray(x).astype(BF16_NP),
        "yh": np.ascontiguousarray(y).astype(BF16_NP),
        "gnx_w": np.asarray(gnx_w, np.float32), "gnx_b": np.asarray(gnx_b, np.float32),
        "gny_w": np.asarray(gny_w, np.float32), "gny_b": np.asarray(gny_b, np.float32),
        "wqT": _pack_w8(wq), "bq": (bq.reshape(CO, P).T * (0.5 * SCALE)).astype(np.float32),
        "wkT": _pack_w8(wk), "bk": bk.reshape(CO, P).T.copy(),
        "wvT": _pack_w8(wv), "bv": bv,
        "woT": np.ascontiguousarray(
            np.asarray(ow, np.float32).T.reshape(CO, P, C).transpose(1, 0, 2)
            .reshape(P, -1)).astype(BF16_NP),
        "bo": np.asarray(ob, np.float32).reshape(CO, P).T.copy(),
    }


def kernel(a, b, gn_a_w, gn_a_b, gn_b_w, gn_b_b,
           qkv_a_w, qkv_a_b, qkv_b_w, qkv_b_b,
           out_a_w, out_a_b, out_b_w, out_b_b):
    a = np.asarray(a); b = np.asarray(b)
    nc = _get_module()
    in_maps = []
    for s in range(N):
        in_maps.append(_core_inputs(a[s], b[s], gn_a_w, gn_a_b, gn_b_w, gn_b_b,
                                    qkv_a_w, qkv_a_b, qkv_b_w, qkv_b_b,
                                    out_a_w, out_a_b))
        in_maps.append(_core_inputs(b[s], a[s], gn_b_w, gn_b_b, gn_a_w, gn_a_b,
                                    qkv_b_w, qkv_b_b, qkv_a_w, qkv_a_b,
                                    out_b_w, out_b_b))
    res = run_bass_kernel_spmd(nc, in_maps, core_ids=list(range(2 * N)))
    out_a = np.stack([res.results[2 * s]["out"] for s in range(N)])
    out_b = np.stack([res.results[2 * s + 1]["out"] for s in range(N)])
    return out_a.astype(np.float32), out_b.astype(np.float32)


# revision 3
# speedup vs baseline: 1.4815x; 1.0453x over previous
"""Cross-attention 1d kernel for Trainium2 (Bass/Tile), SPMD over 8 NeuronCores.

Problem (hardcoded shapes): N=4, C=512, L=2048, H=8, D=64.
  out_x = out_w @ attn(x_norm -> y_norm) + out_b + x   (per core: one (sample,
  direction) pair; 8 cores = 4 samples x 2 directions; no cross-core comms.)

v2 design (vs v1 365us):
  - All projection + attention matmuls in fp8e4m3 with perf_mode=DoubleRow:
    contraction pairs ride the free dims ([K,2,M] lhsT / [K,2,N] rhs), halving
    PE streaming cost twice over (0.5 cyc/row fp8-DR in the cost model).
    Weights are pre-scaled x32 host-side to dodge fp8 subnormals; the 1/32
    rides the (required anyway) PSUM-evacuation ops.
  - Softmax exp is the true bottleneck (H*L*L/128 = 262k engine-columns).
    Split across BOTH pointwise engines: ACT runs real Exp straight to fp8;
    DVE runs Schraudolph's bit-trick exp (score*8*log2e + 56 - 0.35 written
    as uint8 == the fp8e4m3 bit pattern of e^score, max ~9% per-element err,
    irrelevant after averaging over 2048 softmax terms).
  - GroupNorm stats via one-pass bn_stats/bn_aggr (DVE), normalize+fp8-quant
    on GpSimd (frees DVE), cross-partition reduce via tiny ones-matmuls.
  - Softmax denominator via ones-augmented v columns (v|ones swapped per head
    parity so numerators land on the partition half the [c,L] attention
    layout needs); normalize = reciprocal + one multiply per (head, q-chunk).
  - Out-projection stays bf16 (exact weights, 3-term fused evac w/ residual).
"""

import sys

sys.path.insert(0, "/opt/trn_rl_repo")

import numpy as np
import ml_dtypes

import concourse.bass as bass
import concourse.tile as tile
from concourse import bacc, mybir
from concourse.bass import ts
from concourse.bass_utils import run_bass_kernel_spmd

F32 = mybir.dt.float32
BF16 = mybir.dt.bfloat16
FP8 = mybir.dt.float8e4
U8 = mybir.dt.uint8
AF = mybir.ActivationFunctionType
ALU = mybir.AluOpType
DR = mybir.MatmulPerfMode.DoubleRow

N, C, L, H = 4, 512, 2048, 8
D = C // H
EPS = 1e-5
SCALE = float(D) ** -0.5
P = 128
CO = C // P          # 4 channel chunks
LT = L // P          # 16 position tiles of 128
TP = LT // 2         # 8 kt pairs
QC = L // 512        # 4 q chunks of 512
WS = 32.0            # host-side weight pre-scale (fp8 subnormal dodge)

# Schraudolph exp in fp8e4m3 bit space: byte = 8*(x*log2e + 7) - 0.35
LOG2E = 1.4426950408889634
A8 = 8.0 * LOG2E
B8 = 56.0 - 0.35

# exp engine split: ACT true Exp vs DVE Schraudolph, ~59% on ACT
ACT_NUM, ACT_DEN = 38, 64
LEAD = 3             # attn@v trails scores by this many tp slots (global)

BF16_NP = ml_dtypes.bfloat16
FP8_NP = ml_dtypes.float8_e4m3


def _build_module():
    nc = bacc.Bacc("TRN2", target_bir_lowering=False, debug=False, num_devices=8)

    def din(name, shape, dt=F32):
        return nc.dram_tensor(name, list(shape), dt, kind="ExternalInput")

    x_d = din("x", (C, L))            # query-side input (residual, f32)
    xh_d = din("xh", (C, L), BF16)    # bf16 copies: stats/normalize inputs
    yh_d = din("yh", (C, L), BF16)
    gnx_w = din("gnx_w", (C,))
    gnx_b = din("gnx_b", (C,))
    gny_w = din("gny_w", (C,))
    gny_b = din("gny_b", (C,))
    wqT_d = din("wqT", (P, 2 * 2 * C), FP8)        # [p,a,j,c_out] pair-split wT, x32
    wkT_d = din("wkT", (P, 2 * 2 * C), FP8)
    wvT_d = din("wvT", (P, 2 * 2 * C), FP8)        # [p,a,j,c_out], x32
    woT_d = din("woT", (P, CO * C), BF16)          # [p,ko,o] plain transpose
    bq_d = din("bq", (P, CO))         # natural (co p) rows, pre-scaled by SCALE
    bk_d = din("bk", (P, CO))         # natural (co p) rows
    bv_d = din("bv", (C,))
    bo_d = din("bo", (P, CO))         # natural (mo p) rows
    out_d = nc.dram_tensor("out", [C, L], F32, kind="ExternalOutput")

    with tile.TileContext(nc) as tc:
        with (
            tc.tile_pool(name="persist", bufs=1) as pp,
            tc.tile_pool(name="small", bufs=1) as sp,
        ):
            # ---- persistent tiles ----
            q_sb = pp.tile([P, CO, L], FP8)          # [p, co, pos] 8K
            k_sb = pp.tile([P, CO, L], FP8)          # 8K
            vaug = pp.tile([P, LT, H, P], FP8)       # [k, lt, h, v|ones] 16K
            attn = pp.tile([P, CO, L], BF16)         # attention out [c,L] 16K
            wqT = pp.tile([P, 2, 2, CO, P], FP8)     # [p,a,j,mo,o] 2K
            wkT = pp.tile([P, 2, 2, CO, P], FP8)
            wvT = pp.tile([P, 2, 2, C], FP8)
            woT = pp.tile([P, CO, C], BF16)          # 4K

            ones_col = sp.tile([P, 1], F32)
            ones_row = sp.tile([1, P], F32)
            nc.vector.memset(ones_col[:], 1.0)
            nc.vector.memset(ones_row[:], 1.0)
            bq_pc = sp.tile([P, CO], F32)
            bk_pc = sp.tile([P, CO], F32)
            bo_pc = sp.tile([P, CO], F32)
            bv_row = sp.tile([1, C], F32)
            bv_bc = sp.tile([P, C], F32)
            gnw_y_pc = sp.tile([P, CO], F32)
            gnb_y_pc = sp.tile([P, CO], F32)
            gnw_x_pc = sp.tile([P, CO], F32)
            gnb_x_pc = sp.tile([P, CO], F32)

            with tc.tile_pool(name="norm", bufs=1) as npool:
                yn = npool.tile([P, 2, 2, L], FP8)   # [p, a, j, pos] 8K
                xn = npool.tile([P, 2, 2, L], FP8)
                x_sb = npool.tile([P, CO, L], BF16)

                with (
                    tc.tile_pool(name="ps_qkv", bufs=3, space="PSUM") as psQ,
                    tc.tile_pool(name="gn_scr", bufs=2) as gsp,
                    tc.tile_pool(name="psA", bufs=2, space="PSUM") as psA,
                ):
                    def gn_tail(st, w_pc, b_pc, pref, inv):
                        """st [P,2] per-partition (1st, 2nd) moments -> scale/bias;
                        inv converts cross-partition totals to mu / E[x^2]."""
                        tot_p = psA.tile([1, 2], F32, tag="gn_totp")
                        nc.tensor.matmul(tot_p[:], ones_col[:], st[:],
                                         start=True, stop=True)
                        t12 = sp.tile([1, 2], F32, tag=f"{pref}_t12")
                        nc.scalar.copy(t12[:], tot_p[:])
                        bc_p = psA.tile([P, 2], F32, tag="gn_bcp")
                        nc.tensor.matmul(bc_p[:], ones_row[:], t12[:],
                                         start=True, stop=True)
                        mu = sp.tile([P, 1], F32, tag=f"{pref}_mu")
                        nc.vector.tensor_scalar(mu[:], bc_p[:, 0:1], inv, 0.0,
                                                op0=ALU.mult, op1=ALU.add)
                        var = sp.tile([P, 1], F32, tag=f"{pref}_var")
                        nc.vector.tensor_scalar(var[:], bc_p[:, 1:2], inv, EPS,
                                                op0=ALU.mult, op1=ALU.add)
                        musq2 = sp.tile([P, 1], F32, tag=f"{pref}_musq2")
                        nc.vector.tensor_tensor(musq2[:], mu[:], mu[:], ALU.mult)
                        nc.vector.tensor_tensor(var[:], var[:], musq2[:],
                                                ALU.subtract)
                        std = sp.tile([P, 1], F32, tag=f"{pref}_std")
                        nc.scalar.activation(std[:], var[:], AF.Sqrt)
                        rstd = sp.tile([P, 1], F32, tag=f"{pref}_rstd")
                        nc.vector.reciprocal(rstd[:], std[:])
                        nmu = sp.tile([P, 1], F32, tag=f"{pref}_nmu")
                        nc.vector.tensor_scalar(nmu[:], mu[:], -1.0, 0.0,
                                                op0=ALU.mult, op1=ALU.add)
                        scale = sp.tile([P, CO], F32, tag=f"{pref}_scale")
                        bias = sp.tile([P, CO], F32, tag=f"{pref}_bias")
                        nc.vector.tensor_scalar(scale[:], w_pc[:], rstd[:], 0.0,
                                                op0=ALU.mult, op1=ALU.add)
                        nc.vector.scalar_tensor_tensor(bias[:], scale[:], nmu[:],
                                                       b_pc[:],
                                                       op0=ALU.mult, op1=ALU.add)
                        return scale, bias

                    def gn_scale_bias(src_sb, w_pc, b_pc, pref):
                        """y path: one-pass bn_stats (DVE) -> st -> gn_tail."""
                        bns = gsp.tile([P, CO, 4, 6], F32, tag="gn_bns")
                        for co in range(CO):
                            for s4 in range(4):
                                nc.vector.bn_stats(
                                    bns[:, co, s4, :],
                                    src_sb[:, co, ts(s4, 512)])
                        bna = gsp.tile([P, 2], F32, tag="gn_bna")
                        nc.vector.bn_aggr(bna[:], bns[:].rearrange("p a b c -> p (a b c)"))
                        st = sp.tile([P, 2], F32, tag=f"{pref}_st")
                        musq = gsp.tile([P, 1], F32, tag="gn_musq")
                        nc.vector.tensor_tensor(musq[:], bna[:, 0:1], bna[:, 0:1],
                                                ALU.mult)
                        nc.vector.tensor_copy(st[:, 0:1], bna[:, 0:1])
                        nc.vector.tensor_tensor(st[:, 1:2], bna[:, 1:2], musq[:],
                                                ALU.add)
                        return gn_tail(st, w_pc, b_pc, pref, 1.0 / P)

                    def gn_from_sums(sump, sqp, w_pc, b_pc, pref):
                        """x path: per-partition sums + squared-sums -> gn_tail."""
                        st = sp.tile([P, 2], F32, tag=f"{pref}_st")
                        nc.vector.tensor_reduce(st[:, 0:1], sump[:],
                                                axis=mybir.AxisListType.X,
                                                op=ALU.add)
                        nc.vector.tensor_reduce(st[:, 1:2], sqp[:],
                                                axis=mybir.AxisListType.X,
                                                op=ALU.add)
                        return gn_tail(st, w_pc, b_pc, pref, 1.0 / float(C * L))

                    with tc.tile_pool(name="io", bufs=1) as iop:
                        y_sb = iop.tile([P, CO, L], BF16)
                        for co in range(CO):
                            nc.sync.dma_start(
                                y_sb[:, co, :],
                                yh_d[:].rearrange("(co p) l -> p co l", p=P)[:, co, :])
                        for co in range(CO):
                            # second hwdge queue so x streams in parallel with y
                            nc.gpsimd.dma_start(
                                x_sb[:, co, :],
                                xh_d[:].rearrange("(co p) l -> p co l", p=P)[:, co, :])
                        for dr, t in ((gny_w, gnw_y_pc), (gny_b, gnb_y_pc),
                                      (gnx_w, gnw_x_pc), (gnx_b, gnb_x_pc)):
                            nc.sync.dma_start(
                                t[:], dr[:].rearrange("(co p) -> p co", p=P))
                        for dr, t in ((bq_d, bq_pc), (bk_d, bk_pc), (bo_d, bo_pc)):
                            nc.sync.dma_start(t[:], dr[:])
                        nc.sync.dma_start(
                            bv_row[:], bv_d[:].rearrange("(a c) -> a c", a=1))
                        nc.gpsimd.partition_broadcast(bv_bc[:], bv_row[:])
                        # ones half of v_aug: even heads cols D:P, odd 0:D
                        nc.gpsimd.memset(vaug[:, :, 0:H:2, D:P], 1.0)
                        nc.gpsimd.memset(vaug[:, :, 1:H:2, 0:D], 1.0)
                        # bv (x32, bf16) rides the v matmul as a rank-1 update
                        bv32_row = sp.tile([1, C], BF16)
                        nc.gpsimd.tensor_scalar(bv32_row[:], bv_row[:], WS, 0.0,
                                                op0=ALU.mult, op1=ALU.add)
                        ones_1 = sp.tile([1, P], BF16)
                        nc.vector.memset(ones_1[:], 1.0)
                        nc.sync.dma_start(
                            wvT[:], wvT_d[:].rearrange("p (a j c) -> p a j c", a=2, j=2))
                        nc.sync.dma_start(
                            wkT[:], wkT_d[:].rearrange("p (a j t o) -> p a j t o",
                                                       a=2, j=2, t=CO))
                        nc.sync.dma_start(
                            wqT[:], wqT_d[:].rearrange("p (a j t o) -> p a j t o",
                                                       a=2, j=2, t=CO))
                        nc.sync.dma_start(
                            woT[:], woT_d[:].rearrange("p (ko o) -> p ko o", ko=CO))

                        # y stats first (bn_stats on DVE; tiny ACT tail)
                        s_y, b_y = gn_scale_bias(y_sb, gnw_y_pc, gnb_y_pc, "y")
                        for a in range(2):
                            for j in range(2):
                                co = 2 * a + j
                                nc.vector.tensor_scalar(
                                    yn[:, a, j, :], y_sb[:, co, :],
                                    s_y[:, co:co + 1], b_y[:, co:co + 1],
                                    op0=ALU.mult, op1=ALU.add)
                        s_x, b_x = gn_scale_bias(x_sb, gnw_x_pc, gnb_x_pc, "x")
                        for a in range(2):
                            for j in range(2):
                                co = 2 * a + j
                                nc.vector.tensor_scalar(
                                    xn[:, a, j, :], x_sb[:, co, :],
                                    s_x[:, co:co + 1], b_x[:, co:co + 1],
                                    op0=ALU.mult, op1=ALU.add)

                        # vT = (wv32 @ yn + 32 bv)^T / 32 -> vaug v-halves (ACT)
                        for lt in range(LT):
                            vp = psQ.tile([P, C], F32, tag="mm")
                            for a in range(2):
                                nc.tensor.matmul(vp[:], yn[:, a, :, ts(lt, P)],
                                                 wvT[:, a, :, :],
                                                 start=(a == 0), stop=False,
                                                 perf_mode=DR)
                            nc.tensor.matmul(vp[:], ones_1[:], bv32_row[:],
                                             start=False, stop=True)
                            vp_h = vp[:].rearrange("p (h d) -> p h d", d=D)
                            nc.scalar.activation(vaug[:, lt, 0:H:2, 0:D],
                                                 vp_h[:, 0:H:2, :], AF.Copy,
                                                 scale=1.0 / WS)
                            nc.scalar.activation(vaug[:, lt, 1:H:2, D:P],
                                                 vp_h[:, 1:H:2, :], AF.Copy,
                                                 scale=1.0 / WS)




                # ======== attention ========
                with (
                    tc.tile_pool(name="ps_sc", bufs=3, space="PSUM") as ps_sc,
                    tc.tile_pool(name="ps_out", bufs=2, space="PSUM") as ps_out,
                    tc.tile_pool(name="pt_pool", bufs=5) as ptp,
                    tc.tile_pool(name="tail", bufs=2) as tlp,
                ):
                    with (
                        tc.tile_pool(name="outsb", bufs=3) as osp,
                        tc.tile_pool(name="xre", bufs=3) as xrp,
                    ):
                        def outproj_chunk(lc, mo):
                            # out[:, mo, lc] = woT.T @ attn[:, :, lc] + bo + x
                            # (bf16 weights; evac ACT, residual add on Pool)
                            op = ps_out.tile([P, 512], F32, tag="oA",
                                             name=f"op_{lc}_{mo}")
                            for ko in range(CO):
                                nc.tensor.matmul(op[:], woT[:, ko, ts(mo, P)],
                                                 attn[:, ko, ts(lc, 512)],
                                                 start=(ko == 0),
                                                 stop=(ko == CO - 1))
                            o1 = osp.tile([P, 512], F32, tag="o1")
                            nc.scalar.activation(o1[:], op[:], AF.Identity,
                                                 bias=bo_pc[:, mo:mo + 1])
                            xr = xrp.tile([P, 512], F32, tag="xr")
                            nc.sync.dma_start(
                                xr[:],
                                x_d[:].rearrange("(mo p) l -> p mo l", p=P)[:, mo, ts(lc, 512)])
                            o_sb = osp.tile([P, 512], F32, tag="osb")
                            radd = nc.vector if lc == QC - 1 else nc.gpsimd
                            radd.tensor_tensor(o_sb[:], o1[:], xr[:], ALU.add)
                            nc.sync.dma_start(
                                out_d[:].rearrange("(mo p) l -> p mo l", p=P)[:, mo, ts(lc, 512)],
                                o_sb[:])

                        def emit_kproj(mo):
                            # k[:, mo, :] projected just before heads 2mo/2mo+1
                            for lc in range(QC):
                                mmt = ps_sc.tile([P, 2, 512], F32, tag="sc",
                                                 name=f"kmm_{mo}_{lc}")
                                for a in range(2):
                                    nc.tensor.matmul(
                                        mmt[:, 0, :], wkT[:, a, :, mo, :],
                                        yn[:, a, :, ts(lc, 512)],
                                        start=(a == 0), stop=(a == 1),
                                        perf_mode=DR)
                                nc.scalar.activation(
                                    k_sb[:, mo, ts(lc, 512)], mmt[:, 0, :],
                                    AF.Identity, bias=bk_pc[:, mo:mo + 1],
                                    scale=1.0 / WS)

                        def emit_qproj(lc):
                            # q[:, :, lc] projected just-in-time for its qc
                            for mo in range(CO):
                                mmt = ps_sc.tile([P, 2, 512], F32, tag="sc",
                                                 name=f"qmm_{lc}_{mo}")
                                for a in range(2):
                                    nc.tensor.matmul(
                                        mmt[:, 0, :], wqT[:, a, :, mo, :],
                                        xn[:, a, :, ts(lc, 512)],
                                        start=(a == 0), stop=(a == 1),
                                        perf_mode=DR)
                                nc.scalar.activation(
                                    q_sb[:, mo, ts(lc, 512)], mmt[:, 0, :],
                                    AF.Identity, bias=bq_pc[:, mo:mo + 1],
                                    scale=0.5 * SCALE / WS)

                        # Flat software pipeline over all (sweep, tp) slots:
                        # attn@v trails scores/exp by LEAD slots globally, so
                        # sweep boundaries don't bubble the exp engines.
                        sweeps = [(qc, h) for qc in range(QC) for h in range(H)]
                        nslots = len(sweeps) * TP
                        oAs = {}
                        out_todo = []

                        def emit_slot(i):
                            sw, tp = divmod(i, TP)
                            qc, h = sweeps[sw]
                            co_h = h // 2
                            lo = D * (h % 2)
                            scp = ps_sc.tile([P, 2, 512], F32, tag="sc")
                            qv = (q_sb[lo:lo + D, co_h, ts(qc, 512)]
                                  .rearrange("p (a k) -> p a k", a=1)
                                  .broadcast_to((D, 2, 512)))
                            for e in range(2):
                                kt = 2 * tp + e
                                kv = (k_sb[lo:lo + D, co_h, ts(kt, P)]
                                      .rearrange("p (a k) -> p a k", a=1)
                                      .broadcast_to((D, 2, P)))
                                nc.tensor.matmul(
                                    scp[:, e, :], kv, qv,
                                    start=True, stop=True, perf_mode=DR)
                            pt = ptp.tile([P, 2, 512], U8, tag="pt")
                            if (i * ACT_NUM) % ACT_DEN < ACT_NUM:
                                nc.scalar.activation(pt[:].bitcast(FP8),
                                                     scp[:], AF.Exp)
                            else:
                                nc.vector.tensor_scalar(pt[:], scp[:], A8, B8,
                                                        op0=ALU.mult, op1=ALU.add)
                            return pt

                        def emit_av(i, pt):
                            sw, tp = divmod(i, TP)
                            qc, h = sweeps[sw]
                            if tp == 0:
                                oAs[sw] = ps_out.tile([P, 512], F32, tag="oA", name=f"oA_{sw}")
                            nc.tensor.matmul(
                                oAs[sw][:], vaug[:, 2 * tp:2 * tp + 2, h, :],
                                pt[:].bitcast(FP8),
                                start=(tp == 0), stop=(tp == TP - 1),
                                perf_mode=DR)
                            if tp == TP - 1:
                                oA = oAs.pop(sw)
                                lo = D * (h % 2)
                                r = tlp.tile([D, 512], F32, tag="r")
                                nc.vector.reciprocal(r[:],
                                                     oA[P - D - lo:P - lo, :])
                                nc.vector.tensor_tensor(
                                    attn[lo:lo + D, h // 2, ts(qc, 512)],
                                    oA[lo:lo + D, :], r[:], ALU.mult)
                                if h == H - 1:
                                    out_todo.extend(
                                        (lambda lc=qc, mo=mo: outproj_chunk(lc, mo))
                                        for mo in range(CO))

                        pts = {}
                        for i in range(nslots):
                            sw, tp = divmod(i, TP)
                            if tp == 0 and sweeps[sw][0] == 0 \
                                    and sweeps[sw][1] % 2 == 0:
                                emit_kproj(sweeps[sw][1] // 2)
                            if tp == 0 and sweeps[sw][1] == 0:
                                emit_qproj(sweeps[sw][0])
                            pts[i] = emit_slot(i)
                            if i >= LEAD:
                                emit_av(i - LEAD, pts.pop(i - LEAD))
                            if out_todo and (i % TP) == 3:
                                out_todo.pop(0)()
                        for i in range(nslots - LEAD, nslots):
                            emit_av(i, pts.pop(i))
                        for fn in out_todo:
                            fn()

    nc.compile()
    return nc


_NC_CACHE = None


def _get_module():
    global _NC_CACHE
    if _NC_CACHE is None:
        _NC_CACHE = _build_module()
    return _NC_CACHE


def _pack_w8(w):
    """w [C_out, C_in] -> wT pair-split [p, a, j, c_out] fp8, x32."""
    a = (np.asarray(w, np.float64) * WS).T                 # [c_in, c_out]
    a = a.reshape(2, 2, P, C).transpose(2, 0, 1, 3)        # [p, a, j, c_out]
    return np.ascontiguousarray(a.reshape(P, -1)).astype(FP8_NP)


def _core_inputs(x, y, gnx_w, gnx_b, gny_w, gny_b, qw_q, qb_q, qw_kv, qb_kv, ow, ob):
    wq = qw_q[0:C]
    wk = qw_kv[C:2 * C]
    wv = qw_kv[2 * C:3 * C]
    bq = np.asarray(qb_q[0:C], np.float32)
    bk = np.asarray(qb_kv[C:2 * C], np.float32)
    bv = np.asarray(qb_kv[2 * C:3 * C], np.float32)
    return {
        "x": np.ascontiguousarray(x, dtype=np.float32),
        "xh": np.ascontiguousarray(x).astype(BF16_NP),
        "yh": np.ascontiguousarray(y).astype(BF16_NP),
        "gnx_w": np.asarray(gnx_w, np.float32), "gnx_b": np.asarray(gnx_b, np.float32),
        "gny_w": np.asarray(gny_w, np.float32), "gny_b": np.asarray(gny_b, np.float32),
        "wqT": _pack_w8(wq), "bq": (bq.reshape(CO, P).T * (0.5 * SCALE)).astype(np.float32),
        "wkT": _pack_w8(wk), "bk": bk.reshape(CO, P).T.copy(),
        "wvT": _pack_w8(wv), "bv": bv,
        "woT": np.ascontiguousarray(
            np.asarray(ow, np.float32).T.reshape(CO, P, C).transpose(1, 0, 2)
            .reshape(P, -1)).astype(BF16_NP),
        "bo": np.asarray(ob, np.float32).reshape(CO, P).T.copy(),
    }


def kernel(a, b, gn_a_w, gn_a_b, gn_b_w, gn_b_b,
           qkv_a_w, qkv_a_b, qkv_b_w, qkv_b_b,
           out_a_w, out_a_b, out_b_w, out_b_b):
    a = np.asarray(a); b = np.asarray(b)
    nc = _get_module()
    in_maps = []
    for s in range(N):
        in_maps.append(_core_inputs(a[s], b[s], gn_a_w, gn_a_b, gn_b_w, gn_b_b,
                                    qkv_a_w, qkv_a_b, qkv_b_w, qkv_b_b,
                                    out_a_w, out_a_b))
        in_maps.append(_core_inputs(b[s], a[s], gn_b_w, gn_b_b, gn_a_w, gn_a_b,
                                    qkv_b_w, qkv_b_b, qkv_a_w, qkv_a_b,
                                    out_b_w, out_b_b))
    res = run_bass_kernel_spmd(nc, in_maps, core_ids=list(range(2 * N)))
    out_a = np.stack([res.results[2 * s]["out"] for s in range(N)])
    out_b = np.stack([res.results[2 * s + 1]["out"] for s in range(N)])
    return out_a.astype(np.float32), out_b.astype(np.float32)


# revision 4
# speedup vs baseline: 1.4834x; 1.0013x over previous
"""Cross-attention 1d kernel for Trainium2 (Bass/Tile), SPMD over 8 NeuronCores.

Problem (hardcoded shapes): N=4, C=512, L=2048, H=8, D=64.
  out_x = out_w @ attn(x_norm -> y_norm) + out_b + x   (per core: one (sample,
  direction) pair; 8 cores = 4 samples x 2 directions; no cross-core comms.)

v2 design (vs v1 365us):
  - All projection + attention matmuls in fp8e4m3 with perf_mode=DoubleRow:
    contraction pairs ride the free dims ([K,2,M] lhsT / [K,2,N] rhs), halving
    PE streaming cost twice over (0.5 cyc/row fp8-DR in the cost model).
    Weights are pre-scaled x32 host-side to dodge fp8 subnormals; the 1/32
    rides the (required anyway) PSUM-evacuation ops.
  - Softmax exp is the true bottleneck (H*L*L/128 = 262k engine-columns).
    Split across BOTH pointwise engines: ACT runs real Exp straight to fp8;
    DVE runs Schraudolph's bit-trick exp (score*8*log2e + 56 - 0.35 written
    as uint8 == the fp8e4m3 bit pattern of e^score, max ~9% per-element err,
    irrelevant after averaging over 2048 softmax terms).
  - GroupNorm stats via one-pass bn_stats/bn_aggr (DVE), normalize+fp8-quant
    on GpSimd (frees DVE), cross-partition reduce via tiny ones-matmuls.
  - Softmax denominator via ones-augmented v columns (v|ones swapped per head
    parity so numerators land on the partition half the [c,L] attention
    layout needs); normalize = reciprocal + one multiply per (head, q-chunk).
  - Out-projection stays bf16 (exact weights, 3-term fused evac w/ residual).
"""

import sys

sys.path.insert(0, "/opt/trn_rl_repo")

import numpy as np
import ml_dtypes

import concourse.bass as bass
import concourse.tile as tile
from concourse import bacc, mybir
from concourse.bass import ts
from concourse.bass_utils import run_bass_kernel_spmd

F32 = mybir.dt.float32
BF16 = mybir.dt.bfloat16
FP8 = mybir.dt.float8e4
U8 = mybir.dt.uint8
AF = mybir.ActivationFunctionType
ALU = mybir.AluOpType
DR = mybir.MatmulPerfMode.DoubleRow

N, C, L, H = 4, 512, 2048, 8
D = C // H
EPS = 1e-5
SCALE = float(D) ** -0.5
P = 128
CO = C // P          # 4 channel chunks
LT = L // P          # 16 position tiles of 128
TP = LT // 2         # 8 kt pairs
QC = L // 512        # 4 q chunks of 512
WS = 32.0            # host-side weight pre-scale (fp8 subnormal dodge)

# Schraudolph exp in fp8e4m3 bit space: byte = 8*(x*log2e + 7) - 0.35
LOG2E = 1.4426950408889634
A8 = 8.0 * LOG2E
B8 = 56.0 - 0.35

# exp engine split: ACT true Exp vs DVE Schraudolph, ~59% on ACT
ACT_NUM, ACT_DEN = 38, 64
LEAD = 3             # attn@v trails scores by this many tp slots (global)

BF16_NP = ml_dtypes.bfloat16
FP8_NP = ml_dtypes.float8_e4m3


def _build_module():
    nc = bacc.Bacc("TRN2", target_bir_lowering=False, debug=False, num_devices=8)

    def din(name, shape, dt=F32):
        return nc.dram_tensor(name, list(shape), dt, kind="ExternalInput")

    x_d = din("x", (C, L))            # query-side input (residual, f32)
    xh_d = din("xh", (C, L), BF16)    # bf16 copies: stats/normalize inputs
    yh_d = din("yh", (C, L), BF16)
    gnx_w = din("gnx_w", (C,))
    gnx_b = din("gnx_b", (C,))
    gny_w = din("gny_w", (C,))
    gny_b = din("gny_b", (C,))
    wqT_d = din("wqT", (P, 2 * 2 * C), FP8)        # [p,a,j,c_out] pair-split wT, x32
    wkT_d = din("wkT", (P, 2 * 2 * C), FP8)
    wvT_d = din("wvT", (P, 2 * 2 * C), FP8)        # [p,a,j,c_out], x32
    woT_d = din("woT", (P, CO * C), BF16)          # [p,ko,o] plain transpose
    bq_d = din("bq", (P, CO))         # natural (co p) rows, pre-scaled by SCALE
    bk_d = din("bk", (P, CO))         # natural (co p) rows
    bv_d = din("bv", (C,))
    bo_d = din("bo", (P, CO))         # natural (mo p) rows
    out_d = nc.dram_tensor("out", [C, L], F32, kind="ExternalOutput")

    with tile.TileContext(nc) as tc:
        with (
            tc.tile_pool(name="persist", bufs=1) as pp,
            tc.tile_pool(name="small", bufs=1) as sp,
        ):
            # ---- persistent tiles ----
            q_sb = pp.tile([P, CO, L], FP8)          # [p, co, pos] 8K
            k_sb = pp.tile([P, CO, L], FP8)          # 8K
            vaug = pp.tile([P, LT, H, P], FP8)       # [k, lt, h, v|ones] 16K
            attn = pp.tile([P, CO, L], BF16)         # attention out [c,L] 16K
            wqT = pp.tile([P, 2, 2, CO, P], FP8)     # [p,a,j,mo,o] 2K
            wkT = pp.tile([P, 2, 2, CO, P], FP8)
            wvT = pp.tile([P, 2, 2, C], FP8)
            woT = pp.tile([P, CO, C], BF16)          # 4K

            ones_col = sp.tile([P, 1], F32)
            ones_row = sp.tile([1, P], F32)
            nc.vector.memset(ones_col[:], 1.0)
            nc.vector.memset(ones_row[:], 1.0)
            bq_pc = sp.tile([P, CO], F32)
            bk_pc = sp.tile([P, CO], F32)
            bo_pc = sp.tile([P, CO], F32)
            bv_row = sp.tile([1, C], F32)
            bv_bc = sp.tile([P, C], F32)
            gnw_y_pc = sp.tile([P, CO], F32)
            gnb_y_pc = sp.tile([P, CO], F32)
            gnw_x_pc = sp.tile([P, CO], F32)
            gnb_x_pc = sp.tile([P, CO], F32)

            with tc.tile_pool(name="norm", bufs=1) as npool:
                yn = npool.tile([P, 2, 2, L], FP8)   # [p, a, j, pos] 8K
                xn = npool.tile([P, 2, 2, L], FP8)
                x_sb = npool.tile([P, CO, L], BF16)

                with (
                    tc.tile_pool(name="ps_qkv", bufs=3, space="PSUM") as psQ,
                    tc.tile_pool(name="gn_scr", bufs=2) as gsp,
                    tc.tile_pool(name="psA", bufs=2, space="PSUM") as psA,
                ):
                    def gn_tail(st, w_pc, b_pc, pref, inv):
                        """st [P,2] per-partition (1st, 2nd) moments -> scale/bias;
                        inv converts cross-partition totals to mu / E[x^2]."""
                        tot_p = psA.tile([1, 2], F32, tag="gn_totp")
                        nc.tensor.matmul(tot_p[:], ones_col[:], st[:],
                                         start=True, stop=True)
                        t12 = sp.tile([1, 2], F32, tag=f"{pref}_t12")
                        nc.scalar.copy(t12[:], tot_p[:])
                        bc_p = psA.tile([P, 2], F32, tag="gn_bcp")
                        nc.tensor.matmul(bc_p[:], ones_row[:], t12[:],
                                         start=True, stop=True)
                        mu = sp.tile([P, 1], F32, tag=f"{pref}_mu")
                        nc.vector.tensor_scalar(mu[:], bc_p[:, 0:1], inv, 0.0,
                                                op0=ALU.mult, op1=ALU.add)
                        var = sp.tile([P, 1], F32, tag=f"{pref}_var")
                        nc.vector.tensor_scalar(var[:], bc_p[:, 1:2], inv, EPS,
                                                op0=ALU.mult, op1=ALU.add)
                        musq2 = sp.tile([P, 1], F32, tag=f"{pref}_musq2")
                        nc.vector.tensor_tensor(musq2[:], mu[:], mu[:], ALU.mult)
                        nc.vector.tensor_tensor(var[:], var[:], musq2[:],
                                                ALU.subtract)
                        std = sp.tile([P, 1], F32, tag=f"{pref}_std")
                        nc.scalar.activation(std[:], var[:], AF.Sqrt)
                        rstd = sp.tile([P, 1], F32, tag=f"{pref}_rstd")
                        nc.vector.reciprocal(rstd[:], std[:])
                        nmu = sp.tile([P, 1], F32, tag=f"{pref}_nmu")
                        nc.vector.tensor_scalar(nmu[:], mu[:], -1.0, 0.0,
                                                op0=ALU.mult, op1=ALU.add)
                        scale = sp.tile([P, CO], F32, tag=f"{pref}_scale")
                        bias = sp.tile([P, CO], F32, tag=f"{pref}_bias")
                        nc.vector.tensor_scalar(scale[:], w_pc[:], rstd[:], 0.0,
                                                op0=ALU.mult, op1=ALU.add)
                        nc.vector.scalar_tensor_tensor(bias[:], scale[:], nmu[:],
                                                       b_pc[:],
                                                       op0=ALU.mult, op1=ALU.add)
                        return scale, bias

                    def gn_scale_bias(src_sb, w_pc, b_pc, pref):
                        """y path: one-pass bn_stats (DVE) -> st -> gn_tail."""
                        bns = gsp.tile([P, CO, 4, 6], F32, tag="gn_bns")
                        for co in range(CO):
                            for s4 in range(4):
                                nc.vector.bn_stats(
                                    bns[:, co, s4, :],
                                    src_sb[:, co, ts(s4, 512)])
                        bna = gsp.tile([P, 2], F32, tag="gn_bna")
                        nc.vector.bn_aggr(bna[:], bns[:].rearrange("p a b c -> p (a b c)"))
                        st = sp.tile([P, 2], F32, tag=f"{pref}_st")
                        musq = gsp.tile([P, 1], F32, tag="gn_musq")
                        nc.vector.tensor_tensor(musq[:], bna[:, 0:1], bna[:, 0:1],
                                                ALU.mult)
                        nc.vector.tensor_copy(st[:, 0:1], bna[:, 0:1])
                        nc.vector.tensor_tensor(st[:, 1:2], bna[:, 1:2], musq[:],
                                                ALU.add)
                        return gn_tail(st, w_pc, b_pc, pref, 1.0 / P)

                    def gn_from_sums(sump, sqp, w_pc, b_pc, pref):
                        """x path: per-partition sums + squared-sums -> gn_tail."""
                        st = sp.tile([P, 2], F32, tag=f"{pref}_st")
                        nc.vector.tensor_reduce(st[:, 0:1], sump[:],
                                                axis=mybir.AxisListType.X,
                                                op=ALU.add)
                        nc.vector.tensor_reduce(st[:, 1:2], sqp[:],
                                                axis=mybir.AxisListType.X,
                                                op=ALU.add)
                        return gn_tail(st, w_pc, b_pc, pref, 1.0 / float(C * L))

                    with tc.tile_pool(name="io", bufs=1) as iop:
                        y_sb = iop.tile([P, CO, L], BF16)
                        for co in range(CO):
                            nc.sync.dma_start(
                                y_sb[:, co, :],
                                yh_d[:].rearrange("(co p) l -> p co l", p=P)[:, co, :])
                        for co in range(CO):
                            # second hwdge queue so x streams in parallel with y
                            nc.gpsimd.dma_start(
                                x_sb[:, co, :],
                                xh_d[:].rearrange("(co p) l -> p co l", p=P)[:, co, :])
                        for dr, t in ((gny_w, gnw_y_pc), (gny_b, gnb_y_pc),
                                      (gnx_w, gnw_x_pc), (gnx_b, gnb_x_pc)):
                            nc.sync.dma_start(
                                t[:], dr[:].rearrange("(co p) -> p co", p=P))
                        for dr, t in ((bq_d, bq_pc), (bk_d, bk_pc), (bo_d, bo_pc)):
                            nc.sync.dma_start(t[:], dr[:])
                        nc.sync.dma_start(
                            bv_row[:], bv_d[:].rearrange("(a c) -> a c", a=1))
                        nc.gpsimd.partition_broadcast(bv_bc[:], bv_row[:])
                        # ones half of v_aug: even heads cols D:P, odd 0:D
                        nc.gpsimd.memset(vaug[:, :, 0:H:2, D:P], 1.0)
                        nc.gpsimd.memset(vaug[:, :, 1:H:2, 0:D], 1.0)
                        # bv (x32, bf16) rides the v matmul as a rank-1 update
                        bv32_row = sp.tile([1, C], BF16)
                        nc.gpsimd.tensor_scalar(bv32_row[:], bv_row[:], WS, 0.0,
                                                op0=ALU.mult, op1=ALU.add)
                        ones_1 = sp.tile([1, P], BF16)
                        nc.vector.memset(ones_1[:], 1.0)
                        nc.sync.dma_start(
                            wvT[:], wvT_d[:].rearrange("p (a j c) -> p a j c", a=2, j=2))
                        nc.sync.dma_start(
                            wkT[:], wkT_d[:].rearrange("p (a j t o) -> p a j t o",
                                                       a=2, j=2, t=CO))
                        nc.sync.dma_start(
                            wqT[:], wqT_d[:].rearrange("p (a j t o) -> p a j t o",
                                                       a=2, j=2, t=CO))
                        nc.sync.dma_start(
                            woT[:], woT_d[:].rearrange("p (ko o) -> p ko o", ko=CO))

                        # y stats first (bn_stats on DVE; tiny ACT tail)
                        s_y, b_y = gn_scale_bias(y_sb, gnw_y_pc, gnb_y_pc, "y")
                        for a in range(2):
                            for j in range(2):
                                co = 2 * a + j
                                for hh in range(2):
                                    nc.vector.tensor_scalar(
                                        yn[:, a, j, ts(hh, 1024)],
                                        y_sb[:, co, ts(hh, 1024)],
                                        s_y[:, co:co + 1], b_y[:, co:co + 1],
                                        op0=ALU.mult, op1=ALU.add)
                        s_x, b_x = gn_scale_bias(x_sb, gnw_x_pc, gnb_x_pc, "x")
                        for a in range(2):
                            for j in range(2):
                                co = 2 * a + j
                                for hh in range(2):
                                    nc.vector.tensor_scalar(
                                        xn[:, a, j, ts(hh, 1024)],
                                        x_sb[:, co, ts(hh, 1024)],
                                        s_x[:, co:co + 1], b_x[:, co:co + 1],
                                        op0=ALU.mult, op1=ALU.add)

                        # vT = (wv32 @ yn + 32 bv)^T / 32 -> vaug v-halves (ACT)
                        for lt in range(LT):
                            vp = psQ.tile([P, C], F32, tag="mm")
                            for a in range(2):
                                nc.tensor.matmul(vp[:], yn[:, a, :, ts(lt, P)],
                                                 wvT[:, a, :, :],
                                                 start=(a == 0), stop=False,
                                                 perf_mode=DR)
                            nc.tensor.matmul(vp[:], ones_1[:], bv32_row[:],
                                             start=False, stop=True)
                            vp_h = vp[:].rearrange("p (h d) -> p h d", d=D)
                            nc.scalar.activation(vaug[:, lt, 0:H:2, 0:D],
                                                 vp_h[:, 0:H:2, :], AF.Copy,
                                                 scale=1.0 / WS)
                            nc.scalar.activation(vaug[:, lt, 1:H:2, D:P],
                                                 vp_h[:, 1:H:2, :], AF.Copy,
                                                 scale=1.0 / WS)




                # ======== attention ========
                with (
                    tc.tile_pool(name="ps_sc", bufs=3, space="PSUM") as ps_sc,
                    tc.tile_pool(name="ps_out", bufs=2, space="PSUM") as ps_out,
                    tc.tile_pool(name="pt_pool", bufs=5) as ptp,
                    tc.tile_pool(name="tail", bufs=2) as tlp,
                ):
                    with (
                        tc.tile_pool(name="outsb", bufs=3) as osp,
                        tc.tile_pool(name="xre", bufs=3) as xrp,
                    ):
                        def outproj_chunk(lc, mo):
                            # out[:, mo, lc] = woT.T @ attn[:, :, lc] + bo + x
                            # (bf16 weights; evac ACT, residual add on Pool)
                            op = ps_out.tile([P, 512], F32, tag="oA",
                                             name=f"op_{lc}_{mo}")
                            for ko in range(CO):
                                nc.tensor.matmul(op[:], woT[:, ko, ts(mo, P)],
                                                 attn[:, ko, ts(lc, 512)],
                                                 start=(ko == 0),
                                                 stop=(ko == CO - 1))
                            o1 = osp.tile([P, 512], F32, tag="o1")
                            nc.scalar.activation(o1[:], op[:], AF.Identity,
                                                 bias=bo_pc[:, mo:mo + 1])
                            xr = xrp.tile([P, 512], F32, tag="xr")
                            nc.sync.dma_start(
                                xr[:],
                                x_d[:].rearrange("(mo p) l -> p mo l", p=P)[:, mo, ts(lc, 512)])
                            o_sb = osp.tile([P, 512], F32, tag="osb")
                            radd = nc.vector if lc == QC - 1 else nc.gpsimd
                            radd.tensor_tensor(o_sb[:], o1[:], xr[:], ALU.add)
                            nc.sync.dma_start(
                                out_d[:].rearrange("(mo p) l -> p mo l", p=P)[:, mo, ts(lc, 512)],
                                o_sb[:])

                        def emit_kproj(mo):
                            # k[:, mo, :] projected just before heads 2mo/2mo+1
                            for lc in range(QC):
                                mmt = ps_sc.tile([P, 2, 512], F32, tag="sc",
                                                 name=f"kmm_{mo}_{lc}")
                                for a in range(2):
                                    nc.tensor.matmul(
                                        mmt[:, 0, :], wkT[:, a, :, mo, :],
                                        yn[:, a, :, ts(lc, 512)],
                                        start=(a == 0), stop=(a == 1),
                                        perf_mode=DR)
                                nc.scalar.activation(
                                    k_sb[:, mo, ts(lc, 512)], mmt[:, 0, :],
                                    AF.Identity, bias=bk_pc[:, mo:mo + 1],
                                    scale=1.0 / WS)

                        def emit_qproj(lc):
                            # q[:, :, lc] projected just-in-time for its qc
                            for mo in range(CO):
                                mmt = ps_sc.tile([P, 2, 512], F32, tag="sc",
                                                 name=f"qmm_{lc}_{mo}")
                                for a in range(2):
                                    nc.tensor.matmul(
                                        mmt[:, 0, :], wqT[:, a, :, mo, :],
                                        xn[:, a, :, ts(lc, 512)],
                                        start=(a == 0), stop=(a == 1),
                                        perf_mode=DR)
                                nc.scalar.activation(
                                    q_sb[:, mo, ts(lc, 512)], mmt[:, 0, :],
                                    AF.Identity, bias=bq_pc[:, mo:mo + 1],
                                    scale=0.5 * SCALE / WS)

                        # Flat software pipeline over all (sweep, tp) slots:
                        # attn@v trails scores/exp by LEAD slots globally, so
                        # sweep boundaries don't bubble the exp engines.
                        sweeps = [(qc, h) for qc in range(QC) for h in range(H)]
                        nslots = len(sweeps) * TP
                        oAs = {}
                        out_todo = []

                        def emit_slot(i):
                            sw, tp = divmod(i, TP)
                            qc, h = sweeps[sw]
                            co_h = h // 2
                            lo = D * (h % 2)
                            scp = ps_sc.tile([P, 2, 512], F32, tag="sc")
                            qv = (q_sb[lo:lo + D, co_h, ts(qc, 512)]
                                  .rearrange("p (a k) -> p a k", a=1)
                                  .broadcast_to((D, 2, 512)))
                            for e in range(2):
                                kt = 2 * tp + e
                                kv = (k_sb[lo:lo + D, co_h, ts(kt, P)]
                                      .rearrange("p (a k) -> p a k", a=1)
                                      .broadcast_to((D, 2, P)))
                                nc.tensor.matmul(
                                    scp[:, e, :], kv, qv,
                                    start=True, stop=True, perf_mode=DR)
                            pt = ptp.tile([P, 2, 512], U8, tag="pt")
                            if (i * ACT_NUM) % ACT_DEN < ACT_NUM:
                                nc.scalar.activation(pt[:].bitcast(FP8),
                                                     scp[:], AF.Exp)
                            else:
                                nc.vector.tensor_scalar(pt[:], scp[:], A8, B8,
                                                        op0=ALU.mult, op1=ALU.add)
                            return pt

                        def emit_av(i, pt):
                            sw, tp = divmod(i, TP)
                            qc, h = sweeps[sw]
                            if tp == 0:
                                oAs[sw] = ps_out.tile([P, 512], F32, tag="oA", name=f"oA_{sw}")
                            nc.tensor.matmul(
                                oAs[sw][:], vaug[:, 2 * tp:2 * tp + 2, h, :],
                                pt[:].bitcast(FP8),
                                start=(tp == 0), stop=(tp == TP - 1),
                                perf_mode=DR)
                            if tp == TP - 1:
                                oA = oAs.pop(sw)
                                lo = D * (h % 2)
                                r = tlp.tile([D, 512], F32, tag="r")
                                nc.vector.reciprocal(r[:],
                                                     oA[P - D - lo:P - lo, :])
                                nc.vector.tensor_tensor(
                                    attn[lo:lo + D, h // 2, ts(qc, 512)],
                                    oA[lo:lo + D, :], r[:], ALU.mult)
                                if h == H - 1:
                                    out_todo.extend(
                                        (lambda lc=qc, mo=mo: outproj_chunk(lc, mo))
                                        for mo in range(CO))

                        pts = {}
                        for i in range(nslots):
                            sw, tp = divmod(i, TP)
                            if tp == 0 and sweeps[sw][0] == 0 \
                                    and sweeps[sw][1] % 2 == 0:
                                emit_kproj(sweeps[sw][1] // 2)
                            if tp == 0 and sweeps[sw][1] == 0:
                                emit_qproj(sweeps[sw][0])
                            pts[i] = emit_slot(i)
                            if i >= LEAD:
                                emit_av(i - LEAD, pts.pop(i - LEAD))
                            if out_todo and (i % TP) == 3:
                                out_todo.pop(0)()
                        for i in range(nslots - LEAD, nslots):
                            emit_av(i, pts.pop(i))
                        for fn in out_todo:
                            fn()

    nc.compile()
    return nc


_NC_CACHE = None


def _get_module():
    global _NC_CACHE
    if _NC_CACHE is None:
        _NC_CACHE = _build_module()
    return _NC_CACHE


def _pack_w8(w):
    """w [C_out, C_in] -> wT pair-split [p, a, j, c_out] fp8, x32."""
    a = (np.asarray(w, np.float64) * WS).T                 # [c_in, c_out]
    a = a.reshape(2, 2, P, C).transpose(2, 0, 1, 3)        # [p, a, j, c_out]
    return np.ascontiguousarray(a.reshape(P, -1)).astype(FP8_NP)


def _core_inputs(x, y, gnx_w, gnx_b, gny_w, gny_b, qw_q, qb_q, qw_kv, qb_kv, ow, ob):
    wq = qw_q[0:C]
    wk = qw_kv[C:2 * C]
    wv = qw_kv[2 * C:3 * C]
    bq = np.asarray(qb_q[0:C], np.float32)
    bk = np.asarray(qb_kv[C:2 * C], np.float32)
    bv = np.asarray(qb_kv[2 * C:3 * C], np.float32)
    return {
        "x": np.ascontiguousarray(x, dtype=np.float32),
        "xh": np.ascontiguousarray(x).astype(BF16_NP),
        "yh": np.ascontiguousarray(y).astype(BF16_NP),
        "gnx_w": np.asarray(gnx_w, np.float32), "gnx_b": np.asarray(gnx_b, np.float32),
        "gny_w": np.asarray(gny_w, np.float32), "gny_b": np.asarray(gny_b, np.float32),
        "wqT": _pack_w8(wq), "bq": (bq.reshape(CO, P).T * (0.5 * SCALE)).astype(np.float32),
        "wkT": _pack_w8(wk), "bk": bk.reshape(CO, P).T.copy(),
        "wvT": _pack_w8(wv), "bv": bv,
        "woT": np.ascontiguousarray(
            np.asarray(ow, np.float32).T.reshape(CO, P, C).transpose(1, 0, 2)
            .reshape(P, -1)).astype(BF16_NP),
        "bo": np.asarray(ob, np.float32).reshape(CO, P).T.copy(),
    }


def kernel(a, b, gn_a_w, gn_a_b, gn_b_w, gn_b_b,
           qkv_a_w, qkv_a_b, qkv_b_w, qkv_b_b,
           out_a_w, out_a_b, out_b_w, out_b_b):
    a = np.asarray(a); b = np.asarray(b)
    nc = _get_module()
    in_maps = []
    for s in range(N):
        in_maps.append(_core_inputs(a[s], b[s], gn_a_w, gn_a_b, gn_b_w, gn_b_b,
                                    qkv_a_w, qkv_a_b, qkv_b_w, qkv_b_b,
                                    out_a_w, out_a_b))
        in_maps.append(_core_inputs(b[s], a[s], gn_b_w, gn_b_b, gn_a_w, gn_a_b,
                                    qkv_b_w, qkv_b_b, qkv_a_w, qkv_a_b,
                                    out_b_w, out_b_b))
    res = run_bass_kernel_spmd(nc, in_maps, core_ids=list(range(2 * N)))
    out_a = np.stack([res.results[2 * s]["out"] for s in range(N)])
    out_b = np.stack([res.results[2 * s + 1]["out"] for s in range(N)])
    return out_a.astype(np.float32), out_b.astype(np.float32)
